# revision 74
# baseline (speedup 1.0000x reference)
"""Trainium2 Bass kernel for LoRA-fused QKV + RoPE + GQA causal attention + o_proj.

Problem (hardcoded): B=2, S=2048, H=2048, NH=16, KVH=4, HD=128, R=16.

Sharding: 8 cores = batch(2) x kv-head-group(4). Core c handles batch b=c//4,
kv head g=c%4 (q heads 4g..4g+3). Each core computes its 4 heads' attention and
a partial o_proj ([S,H] partial over its 512 o-dims); host sums 4 partials per
batch.

v3 design (fp8-DoubleRow mixed precision over the bf16 v2):
- Transposed space throughout: projections produce qT/kT/vT [d, s], scoresT
  [ks, qs] feeds AV directly, o_proj consumes outT [d, s] stationary. PSUM
  accumulation is always fp32; LoRA folded into W on the host; biases applied
  during PSUM evacuation / rope.
- fp8 (e4m3) DoubleRow matmuls — contracting ks/kt tile PAIRS (256-deep) at
  ~1.5x the bf16 pair rate — for: AV + the all-ones denominator matmul on
  causal chunks 1..3 (attn weights written by exp directly in fp8 with a -3
  shift that cancels in the softmax ratio; v also fp8), and the chunk 1..3
  projections (q/k straight fp8 at x*2 / w*32, dequant 1/64 in the evac Copy
  scale; v as an fp8 hi+lo split-W pair which restores ~bf16 precision).
- Chunk 0 (rows/keys 0..511) stays entirely bf16: early causal rows attend
  few keys, so quantization noise there does not average out. QK and o_proj
  stay bf16 everywhere (128-deep / precision-critical). Measured rel err
  6.5e-3 vs the 2e-2 gate.
- Softmax: no max-subtraction; one ACTIVATE per score-tile pair; denominator
  via ones-stationary matmul (broadcasts across partitions); normalization on
  DVE with reciprocal_approx_fast.
- Scheduling: proj chunks in order [1,2,0,3] so the PE warms up on the small
  fp8 startup set (w8+x8 ~2.9MB) while chunk 0's 5.25MB bf16 set streams
  behind it (early DMA is ramp-limited to ~3MB/20us). o_proj(prev) units are
  interleaved INTO the scalar-bound attention phase (PE chews o_proj matmuls
  while waiting on exp); attention itself is pipelined 2 score-pairs deep.
  Output tiles stream back round-robin on the sync/gpsimd queues (scalar kept
  free for exp during attention).
"""

import hashlib
import numpy as np
import ml_dtypes

import concourse.bass as bass
import concourse.mybir as mybir
import concourse.tile as tile
from concourse import bacc
from concourse.bass_utils import run_bass_kernel_spmd

B, S, H = 2, 2048, 2048
NH, KVH, HD = 16, 4, 128
R = 16
LORA_SCALE = 32.0 / 16.0
ATTN_SCALE = HD ** -0.5

NCORES = 8
GQ = NH // KVH          # 4 q heads per core
NT = GQ + 2             # 6 projection tiles: 4 q heads, 1 k, 1 v
QD = GQ * HD            # 512
CH = 512                # s-chunk width
NCH = S // CH           # 4 s-chunks
KT = H // 128           # 16 contraction k-tiles
NKS = S // 128          # 16 ks tiles
F32 = mybir.dt.float32
F32R = mybir.dt.float32r
BF16 = mybir.dt.bfloat16
F8 = mybir.dt.float8e4
NPBF16 = ml_dtypes.bfloat16
# fp8 attention-weight path (causal chunks j>=1): exp is shifted by EXP_SHIFT
# so e^(s*scale+shift) stays under the e4m3 max (240); the shift scales the
# AV numerator and the ones-denominator identically, so it cancels exactly.
EXP_SHIFT = -3.0

# tile classification codes (host-computed from exp(mask) tiles)
SKIP, PLAIN, MASKED = 0, 1, 2

# content tag: force a fresh NEFF cache key whenever this file changes
with open(__file__, "rb") as _f:
    KTAG = hashlib.sha1(_f.read()).hexdigest()[:10]
K_TAG_INT = int(KTAG, 16)


def _build(cls_grid, causal):
    """Build the SPMD program. cls_grid[i][j] in {SKIP, PLAIN, MASKED} for
    scoresT tile (ks_tile i, qs_chunk j). causal=True generates the diagonal
    mask tiles on device (no emaskT input)."""
    nc = bacc.Bacc("TRN2", target_bir_lowering=False)

    # host-packed for contiguous per-partition DMA:
    # x_pre[c, p, kt, s'] = x[b][s = c*CH+s', h = kt*128+p]  (bf16)
    xT = nc.dram_tensor("xT", [NCH, 128, KT, CH], BF16, kind="ExternalInput")
    # w_pre[p, t, kt, o] = w_eff[h = kt*128+p, t*128+o]  (bf16, LoRA folded)
    wT = nc.dram_tensor("wT", [128, NT, KT, 128], BF16, kind="ExternalInput")
    # fp8 copies for the DoubleRow q/k projections of chunks 1..3:
    # xT8 = e4m3(2*x), w8T = e4m3(32*w_eff[q0..q3,k]); dequant 1/64 on evac
    xT8 = w8T = None
    if causal:
        xT8 = nc.dram_tensor("xT8", [NCH, 128, KT, CH], F8, kind="ExternalInput")
        # 7 t-slices: q0..q3, k (straight) + v_hi, v_lo (split residual pair)
        w8T = nc.dram_tensor("w8T", [128, NT + 1, KT, 128], F8,
                             kind="ExternalInput")
    # [:, 0:NT] plain bias columns; [:, NT:2*NT] partition-swapped (rotate-half)
    biasT = nc.dram_tensor("biasT", [128, 2 * NT], F32, kind="ExternalInput")
    # cache-buster: the PJRT NEFF cache hashes the HLO minus backend_config
    DL = (K_TAG_INT % 97) + 1
    dummy = nc.dram_tensor("cachetag", [1, DL], F32, kind="ExternalInput")
    cosT = nc.dram_tensor("cosT", [HD, S], BF16, kind="ExternalInput")
    ssT = nc.dram_tensor("ssT", [HD, S], BF16, kind="ExternalInput")
    any_masked = any(cls_grid[i][j] == MASKED for i in range(NKS) for j in range(NCH))
    emaskT = None
    if not causal and any_masked:
        emaskT = nc.dram_tensor("emaskT", [S, S], BF16, kind="ExternalInput")
    owT = nc.dram_tensor("owT", [QD, H], BF16, kind="ExternalInput")
    out_p = nc.dram_tensor("out_p", [S, H], BF16, kind="ExternalOutput")

    live_per_j = [[i for i in range(NKS) if cls_grid[i][jj] != SKIP]
                  for jj in range(NCH)]
    masked_per_j = [[i for i in range(NKS) if cls_grid[i][jj] == MASKED]
                    for jj in range(NCH)]
    need = [max(jj, max(live_per_j[jj]) // (CH // 128)) for jj in range(NCH)]
    QCH_BUFS = max(2, max(need[jj] - jj for jj in range(NCH)) + 1)
    if causal:
        QCH_BUFS = 3   # proj order [1,2,0,3]: three q chunks live at once

    with tile.TileContext(nc) as tc:
        from concourse.masks import make_identity
        with tc.tile_pool(name="consts", bufs=1) as consts, \
             tc.tile_pool(name="persist", bufs=1) as persist, \
             tc.tile_pool(name="qch", bufs=QCH_BUFS) as qch_pool, \
             tc.tile_pool(name="outp", bufs=2) as outp_pool, \
             tc.tile_pool(name="p1", bufs=5) as p1, \
             tc.tile_pool(name="xch", bufs=2) as xch_pool, \
             tc.tile_pool(name="att", bufs=5) as att_pool, \
             tc.tile_pool(name="stgp", bufs=8) as stgp, \
             tc.tile_pool(name="fin", bufs=2) as fin, \
             tc.tile_pool(name="pp_pair", bufs=2, space="PSUM") as pp_pair, \
             tc.tile_pool(name="pp_o", bufs=2, space="PSUM") as pp_o, \
             tc.tile_pool(name="pp_sum", bufs=1, space="PSUM") as pp_sum, \
             tc.tile_pool(name="pp_t", bufs=1, space="PSUM") as pp_t:

            # causal proj order [1, 2, 0, 3]: the first two chunks run on the
            # small fp8 weight/activation set (w8 1.8MB + x8 1MB each) so the
            # PE starts ~4x sooner than the 5.25MB bf16 chunk-0 set allows;
            # chunk 0's bf16 weights stream in behind them.
            proj_order = [1, 2, 0, 3] if causal else list(range(NCH))

            # gpsimd-generated const scratch emitted BEFORE any DMA
            # descriptor hits the gpsimd queue: the queue stalls ~20us on
            # DMA credits, which otherwise gates permb (rope stationary,
            # needed ~13us) and identb (v transpose, needed ~21us)
            ident_f = consts.tile([128, 128], F32, tag="ident_f")
            nc.gpsimd.memset(ident_f, 0.0)
            nc.gpsimd.affine_select(
                out=ident_f, in_=ident_f,
                compare_op=mybir.AluOpType.not_equal,
                fill=1.0, base=0, channel_multiplier=1, pattern=[[-1, 128]],
            )
            # half-rotation permutation: perm[p, q] = 1 iff q == (p+64)%128.
            # Used as a matmul stationary to compute rotate-half on the PE.
            perm_f = consts.tile([128, 128], F32, tag="perm_f")
            nc.gpsimd.memset(perm_f, 0.0)
            for base in (64, -64):
                nc.gpsimd.affine_select(
                    out=perm_f, in_=perm_f,
                    compare_op=mybir.AluOpType.not_equal,
                    fill=1.0, base=base, channel_multiplier=1,
                    pattern=[[-1, 128]],
                )

            x_tiles = {}
            x8_tiles = {}

            def emit_x_dma(c, fine=False):
                if causal and c >= 1:
                    # chunks >=1 project entirely from x8 (v uses split-W fp8)
                    x8_c = xch_pool.tile([128, KT, CH], F8, tag="x8_c",
                                         bufs=3, name=f"x8_{c}")
                    x8_tiles[c] = x8_c
                    if fine:
                        for kp in range(KT // 2):
                            q = nc.sync if kp % 2 == 0 else nc.gpsimd
                            q.dma_start(out=x8_c[:, bass.ds(kp * 2, 2), :],
                                        in_=xT8[c, :, bass.ds(kp * 2, 2), :])
                    else:
                        nc.sync.dma_start(out=x8_c[:, bass.ds(0, 8), :],
                                          in_=xT8[c, :, bass.ds(0, 8), :])
                        nc.gpsimd.dma_start(out=x8_c[:, bass.ds(8, 8), :],
                                            in_=xT8[c, :, bass.ds(8, 8), :])
                    return
                bufs = 1 if causal else 2
                x_c = xch_pool.tile([128, KT, CH], BF16, tag="x_c",
                                    bufs=bufs, name=f"x_{c}")
                x_tiles[c] = x_c
                if fine:
                    # 16 single-kt pieces: first matmul starts after 0.13MB
                    for kt in range(KT):
                        q = nc.sync if kt % 2 == 0 else nc.gpsimd
                        q.dma_start(out=x_c[:, bass.ds(kt, 1), :],
                                    in_=xT[c, :, bass.ds(kt, 1), :])
                else:
                    qs = [nc.sync, nc.gpsimd, nc.sync, nc.gpsimd]
                    for kq in range(4):
                        qs[kq].dma_start(out=x_c[:, bass.ds(kq * 4, 4), :],
                                         in_=xT[c, :, bass.ds(kq * 4, 4), :])

            cs_tiles = {}

            def emit_cs_dma(c, q=None):
                q = q or nc.sync
                sl = bass.ds(c * CH, CH)
                cos_c = xch_pool.tile([128, CH], BF16, tag="cos_c", bufs=3,
                                      name=f"cos_{c}")
                q.dma_start(out=cos_c, in_=cosT[:, sl])
                ss_c = xch_pool.tile([128, CH], BF16, tag="ss_c", bufs=3,
                                     name=f"ss_{c}")
                q.dma_start(out=ss_c, in_=ssT[:, sl])
                cs_tiles[c] = (cos_c, ss_c)

            # ---- startup DMAs ----
            w_sb = persist.tile([128, NT, KT, 128], BF16, tag="w_sb")
            w8_sb = (persist.tile([128, NT + 1, KT, 128], F8, tag="w8_sb",
                                  name="w8_sb") if causal else None)
            ow_sb = persist.tile([128, GQ, H], BF16, tag="ow_sb")
            ow_done = [False]
            if causal:
                # critical path: w8 (t-need order, t0 split fine) on scalar;
                # x8(1) fine on sync/gpsimd
                for hp in range(2):
                    nc.scalar.dma_start(out=w8_sb[:, 0, bass.ds(hp * 8, 8), :],
                                        in_=w8T[:, 0, bass.ds(hp * 8, 8), :])
                emit_x_dma(proj_order[0], fine=True)
                for t in [1, GQ, 2, 3]:
                    nc.scalar.dma_start(out=w8_sb[:, t, :, :],
                                        in_=w8T[:, t, :, :])
                nc.scalar.dma_start(out=w8_sb[:, bass.ds(NT - 1, 2), :, :],
                                    in_=w8T[:, bass.ds(NT - 1, 2), :, :])
                bias_sb = consts.tile([128, 2 * NT], F32, tag="bias_sb")
                nc.gpsimd.dma_start(out=bias_sb, in_=biasT[:, :])
                emit_cs_dma(proj_order[0], q=nc.gpsimd)
                # second fp8 chunk + its rope tables
                emit_x_dma(proj_order[1])
                emit_cs_dma(proj_order[1], q=nc.gpsimd)
                # chunk 0's bf16 set streams in behind (t-need order). x0
                # goes AHEAD of the later w tiles on sync/gpsimd: proj(0)'s
                # first t-group contracts over the WHOLE x0 chunk, while
                # w GQ/2/3/v aren't consumed until later t-groups
                for kq in range(4):
                    nc.scalar.dma_start(out=w_sb[:, 0, bass.ds(kq * 4, 4), :],
                                        in_=wT[:, 0, bass.ds(kq * 4, 4), :])
                nc.scalar.dma_start(out=w_sb[:, 1, :, :], in_=wT[:, 1, :, :])
                emit_x_dma(0)
                nc.sync.dma_start(out=w_sb[:, GQ, :, :], in_=wT[:, GQ, :, :])
                nc.gpsimd.dma_start(out=w_sb[:, 2, :, :], in_=wT[:, 2, :, :])
                nc.sync.dma_start(out=w_sb[:, 3, :, :], in_=wT[:, 3, :, :])
                nc.gpsimd.dma_start(out=w_sb[:, NT - 1, :, :],
                                    in_=wT[:, NT - 1, :, :])
                emit_cs_dma(0)
            else:
                # t0 in 4 fine pieces so the first matmul starts after ~0.16MB
                for kq in range(4):
                    nc.scalar.dma_start(out=w_sb[:, 0, bass.ds(kq * 4, 4), :],
                                        in_=wT[:, 0, bass.ds(kq * 4, 4), :])
                emit_x_dma(0, fine=True)
                for t in [1, GQ, 2]:   # proj t-need order
                    nc.scalar.dma_start(out=w_sb[:, t, :, :], in_=wT[:, t, :, :])
                nc.sync.dma_start(out=w_sb[:, 3, :, :], in_=wT[:, 3, :, :])
                nc.gpsimd.dma_start(out=w_sb[:, NT - 1, :, :],
                                    in_=wT[:, NT - 1, :, :])
                emit_cs_dma(0)
                bias_sb = consts.tile([128, 2 * NT], F32, tag="bias_sb")
                nc.gpsimd.dma_start(out=bias_sb, in_=biasT[:, :])
            dummy_sb = consts.tile([1, 128], F32, tag="dummy_sb")
            nc.gpsimd.dma_start(out=dummy_sb[:, 0:DL], in_=dummy[:, :])

            # ---- small constants ----
            # full 128-col all-ones stationary: the denominator matmul then
            # broadcasts the column sums across all 128 PSUM partitions (no
            # gpsimd partition_broadcast needed) and keeps LDWEIGHTS
            # pull-ahead working (no col_grp restriction)
            ones_mat = consts.tile([128, 128], BF16, tag="ones_mat")
            nc.vector.memset(ones_mat, 1.0)
            # fp8 all-ones stationary PAIR for DoubleRow denominator matmuls
            ones8 = consts.tile([128, 2, 128], F8, tag="ones8")
            nc.vector.memset(ones8, 1.0)
            # per-partition bias column holding EXP_SHIFT for the fp8 exp
            eshift = consts.tile([128, 1], F32, tag="eshift")
            nc.vector.memset(eshift, EXP_SHIFT)
            identb = consts.tile([128, 128], BF16, tag="identb")
            nc.vector.tensor_copy(out=identb, in_=ident_f)
            permb = consts.tile([128, 128], BF16, tag="permb")
            nc.vector.tensor_copy(out=permb, in_=perm_f)

            # causal: single [128,128] lower-triangle mask; diag tiles are
            # processed as singles restricted to their live column span
            # [128*d, CH), where only the first 128 columns are triangular
            tri = None
            if causal:
                scratch = consts.tile([128, 128], F32, tag="tri_scratch")
                nc.gpsimd.memset(scratch, 0.0)
                nc.gpsimd.affine_select(
                    out=scratch, in_=scratch,
                    compare_op=mybir.AluOpType.is_gt,
                    fill=1.0,
                    base=0,
                    channel_multiplier=1,
                    pattern=[[-1, 128]],
                )
                tri = consts.tile([128, 128], BF16, tag="tri")
                nc.vector.tensor_copy(out=tri, in_=scratch)

            # not-yet-fetched chunks are prefetched lazily (first attention)
            # so they don't steal startup fabric bandwidth
            n_startup = 3 if causal else 1
            to_fetch = [c for c in proj_order[n_startup:]]

            def emit_prefetch():
                if to_fetch:
                    c = to_fetch.pop(0)
                    emit_x_dma(c)
                    emit_cs_dma(c)
                if not ow_done[0]:
                    ow_done[0] = True
                    nc.gpsimd.dma_start(
                        out=ow_sb, in_=owT.rearrange("(g p) n -> p g n", p=128))

            # ---- persistent tiles ----
            kT_full = persist.tile([128, S], BF16, tag="kT_full")
            v_nat = persist.tile([128, NKS, 128], BF16, tag="v_nat")  # [ks, tile, d]
            v_nat8 = (persist.tile([128, NKS, 128], F8, tag="v_nat8",
                                   name="v_nat8") if causal else None)

            out_dma_q = [nc.sync, nc.gpsimd]
            out_dma_n = [0]

            def oproj_units(args, final=False, during_attn=False, last=False):
                """One unit per [128,CH] output tile: 4 accum matmuls + evac +
                DMA. during_attn keeps the scalar engine free for exp; in the
                LAST attention chunk the late units alternate onto scalar
                (its exps finish early there) so DVE stays free for the head
                finalizes the final o_proj depends on."""
                cc, outT_ch = args
                if final:
                    qlist = [nc.sync, nc.gpsimd, nc.scalar]
                elif during_attn:
                    # keep the scalar queue free for exp during attention
                    qlist = [nc.sync, nc.gpsimd]
                else:
                    qlist = [nc.gpsimd, nc.scalar]

                def unit(st4, nch, idx):
                    ssl = bass.ds(st4 * 128, 128)
                    dsl = bass.ds((cc * (CH // 128) + st4) * 128, 128)
                    pop_deferred_v()
                    nsl = bass.ds(nch * CH, CH)
                    ps3 = pp_pair.tile([128, 2, CH], F32, tag="pair", name="ps3")
                    g = st4 * NCH + nch
                    half = g % 2
                    for h in range(GQ):
                        nc.tensor.matmul(ps3[:, half, :], outT_ch[h][:, ssl],
                                         ow_sb[:, h, nsl],
                                         start=(h == 0), stop=(h == GQ - 1))
                    stg = stgp.tile([128, CH], BF16, tag="stg")
                    if final:
                        # split evac across both engines + 2 DMAs: shortens
                        # the end-of-kernel evac/drain critical chain
                        lo, hi = bass.ds(0, CH // 2), bass.ds(CH // 2, CH // 2)
                        nc.vector.tensor_copy(out=stg[:, lo],
                                              in_=ps3[:, half, lo])
                        nc.scalar.activation(out=stg[:, hi],
                                             in_=ps3[:, half, hi],
                                             func=mybir.ActivationFunctionType.Copy)
                        for piece, csl in ((lo, bass.ds(nch * CH, CH // 2)),
                                           (hi, bass.ds(nch * CH + CH // 2,
                                                        CH // 2))):
                            q = qlist[out_dma_n[0] % len(qlist)]
                            out_dma_n[0] += 1
                            q.dma_start(out=out_p[dsl, csl], in_=stg[:, piece])
                        return
                    use_scalar = ((not during_attn and g % 2 == 1)
                                  or (during_attn and last and idx >= 10
                                      and idx % 2 == 1))
                    if use_scalar:
                        nc.scalar.activation(out=stg, in_=ps3[:, half, :],
                                             func=mybir.ActivationFunctionType.Copy)
                    else:
                        nc.vector.tensor_copy(out=stg, in_=ps3[:, half, :])
                    q = qlist[out_dma_n[0] % len(qlist)]
                    out_dma_n[0] += 1
                    q.dma_start(out=out_p[dsl, nsl], in_=stg)

                return [(lambda st4=st4, nch=nch, idx=idx: unit(st4, nch, idx))
                        for idx, (st4, nch) in enumerate(
                            (s, n) for s in range(CH // 128)
                            for n in range(NCH))]

            def emit_oproj(args, final=False):
                for u in oproj_units(args, final=final):
                    u()

            q_chunks = {}
            deferred_v = []

            def pop_deferred_v():
                """Emit ONE pending v transpose (or nothing)."""
                if not deferred_v:
                    return
                c, vT_c, i4 = deferred_v.pop(0)
                i = c * (CH // 128) + i4
                ps_t = pp_t.tile([128, 128], BF16, tag="ps_t", name="ps_t")
                nc.tensor.transpose(ps_t, vT_c[:, bass.ds(i4 * 128, 128)], identb)
                nc.vector.tensor_copy(out=v_nat[:, i, :], in_=ps_t)
                if v_nat8 is not None:
                    nc.vector.tensor_copy(out=v_nat8[:, i, :], in_=ps_t)

            def flush_deferred_v():
                while deferred_v:
                    pop_deferred_v()

            def emit_proj(c, evac_vec=False):
                x_c = x_tiles.get(c)
                cos_c, ss_c = cs_tiles[c]
                sl = bass.ds(c * CH, CH)

                q_ch = [qch_pool.tile([128, CH], BF16, tag=f"qch{h}", name=f"qch{h}_{c}")
                        for h in range(GQ)]
                q_chunks[c] = q_ch

                pending_rope = []

                def flush_rope():
                    # rotate-half via PE permutation matmul, then rope on DVE:
                    # dst = (raw+b)*cos + (perm@raw + swap(b))*ss
                    for t, raw in pending_rope:
                        bias_col = bias_sb[:, t:t + 1]
                        bias_sw = bias_sb[:, NT + t:NT + t + 1]
                        ps_sw = pp_t.tile([128, CH], F32, tag="ps_t", name="ps_sw")
                        nc.tensor.matmul(ps_sw, permb, raw, start=True, stop=True)
                        sw = p1.tile([128, CH], BF16, tag="sw")
                        nc.vector.scalar_tensor_tensor(
                            out=sw, in0=ps_sw, scalar=bias_sw, in1=ss_c,
                            op0=mybir.AluOpType.add, op1=mybir.AluOpType.mult)
                        dst = q_ch[t] if t < GQ else kT_full[:, sl]
                        nc.vector.scalar_tensor_tensor(
                            out=dst, in0=raw, scalar=bias_col, in1=cos_c,
                            op0=mybir.AluOpType.add, op1=mybir.AluOpType.mult)
                        nc.vector.tensor_add(dst, dst, sw)
                    pending_rope.clear()

                def evac_copy(dst, src, dq):
                    # evac_vec: during the startup phase the scalar queue is
                    # blocked on DMA-descriptor credit stalls — evacuate on
                    # the vector engine so the PE pipeline isn't gated on it
                    if evac_vec:
                        if dq:
                            nc.vector.tensor_scalar_mul(dst, src, 1.0 / 64.0)
                        else:
                            nc.vector.tensor_copy(out=dst, in_=src)
                    else:
                        nc.scalar.activation(out=dst, in_=src,
                                             func=mybir.ActivationFunctionType.Copy,
                                             scale=(1.0 / 64.0) if dq else 1.0)

                def evac_tile(t, ps, half, dq=False):
                    if t == NT - 1:   # v (bias folded on host); transpose deferred
                        vT_c = p1.tile([128, CH], BF16, tag="vT_c")
                        evac_copy(vT_c, ps[:, half, :], dq)
                        for i4 in range(CH // 128):
                            deferred_v.append((c, vT_c, i4))
                    else:
                        raw = p1.tile([128, CH], BF16, tag="raw")
                        evac_copy(raw, ps[:, half, :], dq)
                        pending_rope.append((t, raw))

                # fp8 DoubleRow projections for chunks >= 1: q/k use straight
                # fp8 weights (their quantization noise averages out through
                # softmax over >=512 keys); v uses a hi+lo split-W pair, which
                # restores ~bf16 weight precision at the same PE cost
                fp8p = w8_sb is not None and c >= 1

                # t order: q0, q1, k, q2, q3, v; each tile's rope flushes one
                # MM-group later so the PSUM evacuation copy is never waited on
                for n, t in enumerate([0, 1, GQ, 2, 3, NT - 1]):
                    ps = pp_pair.tile([128, 2, CH], F32, tag="pair",
                                      name="ps_p")
                    half = n % 2
                    if fp8p:
                        x8_c = x8_tiles[c]
                        slices = [t] if t != NT - 1 else [NT - 1, NT]
                        total = len(slices) * (KT // 2)
                        nmm = 0
                        for w8t in slices:
                            for kp in range(KT // 2):
                                nc.tensor.matmul(
                                    ps[:, half, :],
                                    w8_sb[:, w8t, bass.ds(2 * kp, 2), :],
                                    x8_c[:, bass.ds(2 * kp, 2), :],
                                    start=(nmm == 0), stop=(nmm == total - 1),
                                    perf_mode=mybir.MatmulPerfMode.DoubleRow)
                                nmm += 1
                    else:
                        for kt in range(KT):
                            nc.tensor.matmul(ps[:, half, :], w_sb[:, t, kt, :],
                                             x_c[:, kt, :],
                                             start=(kt == 0), stop=(kt == KT - 1))
                    flush_rope()
                    pop_deferred_v()
                    evac_tile(t, ps, half, dq=fp8p)
                flush_rope()

            def emit_attention(j, bg=()):
                """Emits the attention for chunk j with a 2-pair software
                pipeline across head boundaries. bg: list of o_proj unit
                emitters interleaved into the (scalar-bound) attention phase
                so the PE chews o_proj matmuls while waiting on exp."""
                flush_deferred_v()
                sl = bass.ds(j * CH, CH)
                live = live_per_j[j]
                masked = set(masked_per_j[j])
                # entry list: (i0, i1 or None, qlo, diag). Causal: plain tiles
                # paired full-width; diag tiles as singles restricted to their
                # live span [qlo, CH) with a [128,128] triangle mask on the
                # first 128 live columns.
                ents = []   # (i0, i1|None, s0, s1|None, diag)
                if causal:
                    plain = [i for i in live if i not in masked]
                    for n in range(0, len(plain), 2):
                        i1 = plain[n + 1] if n + 1 < len(plain) else None
                        ents.append((plain[n], i1, 0, 0 if i1 is not None else None,
                                     False))
                    dg = sorted(masked)
                    for n in range(0, len(dg), 2):
                        i0 = dg[n]
                        i1 = dg[n + 1] if n + 1 < len(dg) else None
                        s0 = 128 * (i0 - 4 * j)
                        s1 = 128 * (i1 - 4 * j) if i1 is not None else None
                        ents.append((i0, i1, s0, s1, True))
                else:
                    for n in range(0, len(live), 2):
                        i1 = live[n + 1] if n + 1 < len(live) else None
                        ents.append((live[n], i1, 0, 0 if i1 is not None else None,
                                     False))

                outT_ch = [outp_pool.tile([128, CH], BF16, tag=f"outT{h}",
                                          name=f"outT{h}_{j}") for h in range(GQ)]
                q_ch = q_chunks[j]

                # global pipeline over (head, entry)
                work = [(h, e) for h in range(GQ) for e in ents]
                NP = len(work)
                attns = {}

                # fp8 DoubleRow path: causal chunks j>=1 (every row there has
                # >=512 live keys, so fp8 quantization noise on the attention
                # weights and v averages out; chunk 0 stays bf16)
                fp8 = causal and j >= 1

                def emit_qk_exp(n):
                    h, (i0, i1, s0, s1, diag) = work[n]
                    qh = q_ch[h]
                    sp0 = bass.ds(s0, CH - s0)
                    pr = pp_pair.tile([128, 2, CH], F32, tag="pair", name="pr")
                    nc.tensor.matmul(pr[:, 0, sp0],
                                     kT_full[:, bass.ds(i0 * 128, 128)],
                                     qh[:, sp0], start=True, stop=True)
                    if i1 is not None:
                        sp1 = bass.ds(s1, CH - s1)
                        nc.tensor.matmul(pr[:, 1, sp1],
                                         kT_full[:, bass.ds(i1 * 128, 128)],
                                         qh[:, sp1], start=True, stop=True)
                    if fp8:
                        attn = att_pool.tile([128, 2, CH], F8, tag="attn8")
                    else:
                        attn = att_pool.tile([128, 2, CH], BF16, tag="attn")
                    # one rectangular exp over both halves from s0 (s0 <= s1;
                    # half1's [s0,s1) region is garbage: never read in the
                    # bf16 path, zeroed below before the DoubleRow matmul)
                    src = pr[:, :, sp0] if i1 is not None else pr[:, 0:1, sp0]
                    dst = attn[:, :, sp0] if i1 is not None else attn[:, 0:1, sp0]
                    nc.scalar.activation(out=dst, in_=src,
                                         func=mybir.ActivationFunctionType.Exp,
                                         scale=float(ATTN_SCALE),
                                         bias=eshift[:, :] if fp8 else 0.0)
                    if diag:
                        nc.vector.tensor_mul(attn[:, 0, bass.ds(s0, 128)],
                                             attn[:, 0, bass.ds(s0, 128)], tri)
                        if i1 is not None:
                            nc.vector.tensor_mul(attn[:, 1, bass.ds(s1, 128)],
                                                 attn[:, 1, bass.ds(s1, 128)], tri)
                            if fp8 and s1 > s0:
                                # zero half1's dead span so DoubleRow can feed
                                # both halves over the shared [s0, CH) span
                                nc.vector.memset(
                                    attn[:, 1, bass.ds(s0, s1 - s0)], 0.0)
                    elif (i0 in masked) or (i1 in masked):
                        mt = att_pool.tile([128, 2, CH], BF16, tag="m_tile",
                                           bufs=3, name=f"mt_{j}_{h}_{i0}")
                        nc.gpsimd.dma_start(out=mt[:, 0, :],
                                            in_=emaskT[bass.ds(i0 * 128, 128), sl])
                        if i1 is not None:
                            nc.gpsimd.dma_start(
                                out=mt[:, 1, :],
                                in_=emaskT[bass.ds(i1 * 128, 128), sl])
                        msl = attn if i1 is not None else attn[:, 0:1, :]
                        mm = mt if i1 is not None else mt[:, 0:1, :]
                        nc.vector.tensor_mul(msl, msl, mm)
                    attns[n] = attn

                def emit_av(n):
                    h, (i0, i1, s0, s1, diag) = work[n]
                    attn = attns.pop(n)
                    sp0 = bass.ds(s0, CH - s0)
                    pidx = n % len(ents)
                    first, last = (pidx == 0), (pidx == len(ents) - 1)
                    ps_o = st_o[h]
                    ps_sum = st_sum[h]
                    if fp8 and i1 is not None:
                        # one DoubleRow matmul contracts both ks tiles of the
                        # pair (i1 == i0+1 so the v pair is contiguous)
                        assert i1 == i0 + 1
                        nc.tensor.matmul(ps_o[:, sp0],
                                         v_nat8[:, bass.ds(i0, 2), :],
                                         attn[:, :, sp0],
                                         start=first, stop=last,
                                         perf_mode=mybir.MatmulPerfMode.DoubleRow)
                        nc.tensor.matmul(ps_sum[:, sp0], ones8,
                                         attn[:, :, sp0],
                                         start=first, stop=last,
                                         perf_mode=mybir.MatmulPerfMode.DoubleRow)
                        if last:
                            finalize(h)
                        return
                    v0 = v_nat8[:, i0, :] if fp8 else v_nat[:, i0, :]
                    o0 = ones8[:, 0, :] if fp8 else ones_mat
                    nc.tensor.matmul(ps_o[:, sp0], v0,
                                     attn[:, 0, sp0],
                                     start=first, stop=(last and i1 is None))
                    nc.tensor.matmul(ps_sum[:, sp0], o0,
                                     attn[:, 0, sp0],
                                     start=first, stop=(last and i1 is None))
                    if i1 is not None:
                        sp1 = bass.ds(s1, CH - s1)
                        nc.tensor.matmul(ps_o[:, sp1], v_nat[:, i1, :],
                                         attn[:, 1, sp1],
                                         start=False, stop=last)
                        nc.tensor.matmul(ps_sum[:, sp1], ones_mat,
                                         attn[:, 1, sp1],
                                         start=False, stop=last)
                    if last:
                        finalize(h)

                st_o, st_sum = {}, {}

                def start_head(h):
                    st_o[h] = pp_o.tile([128, CH], F32, tag="ps_o", name="ps_o")
                    st_sum[h] = pp_sum.tile([128, CH], F32, tag="ps_sum",
                                            name="ps_sum")

                def finalize(h):
                    # ps_sum already holds the denominator in every partition
                    recip = fin.tile([128, CH], F32, tag="recip")
                    nc.vector.reciprocal_approx_fast(out=recip, in_=st_sum[h])
                    nc.vector.tensor_mul(outT_ch[h], st_o[h], recip)

                PIPE = 4
                npairs = len(ents)
                bg = list(bg)
                bgi = [0]

                def pump_bg():
                    if bgi[0] < len(bg):
                        bg[bgi[0]]()
                        bgi[0] += 1

                every = max(1, NP // len(bg)) if bg else 0
                for n in range(NP):
                    if n % npairs == 0:
                        start_head(work[n][0])
                    emit_qk_exp(n)
                    # bg unit BEFORE the AV: the PE queue is in-order, so the
                    # o_proj matmuls fill the wait for exp(n-PIPE) instead of
                    # sitting blocked behind the stalled AV
                    if bg and n >= PIPE and n % every == 0:
                        pump_bg()
                    if n >= PIPE:
                        emit_av(n - PIPE)
                # flush leftover bg units BEFORE the AV tail so their evacs
                # precede the last heads' finalize on the engine queues
                while bgi[0] < len(bg):
                    pump_bg()
                for n in range(max(0, NP - PIPE), NP):
                    emit_av(n)
                return outT_ch

            # ---- main fused loop: proj chunks in proj_order; attention(j)
            # as soon as chunks 0..need[j] are projected, with the previous
            # o_proj's units interleaved into the scalar-bound attention ----
            pending_oproj = None
            done_proj = set()
            attended = set()
            for pi, p in enumerate(proj_order):
                emit_proj(p, evac_vec=causal and pi < 2)
                done_proj.add(p)
                first_att = True
                for j in range(NCH):
                    if j in attended:
                        continue
                    if all(k in done_proj for k in range(need[j] + 1)):
                        if first_att:
                            # prefetch next chunk's inputs during attention
                            emit_prefetch()
                            first_att = False
                        bg = (oproj_units(pending_oproj, during_attn=True,
                                          last=(j == NCH - 1))
                              if pending_oproj is not None else [])
                        pending_oproj = (j, emit_attention(j, bg))
                        attended.add(j)
                if first_att:
                    emit_prefetch()
                    if pending_oproj is not None:
                        emit_oproj(pending_oproj)
                        pending_oproj = None

            if pending_oproj is not None:
                emit_oproj(pending_oproj, final=True)
                pending_oproj = None

    nc.finalize()
    return nc


_cache = {}


def _get_program(key, cls_grid, causal):
    if key not in _cache:
        _cache[key] = _build(cls_grid, causal)
    return _cache[key]


def _classify(em_t):
    """em_t: exp(mask).T [S, S] (ks, qs). Returns tuple-of-tuples class grid
    [NKS][NCH]."""
    grid = []
    for i in range(NKS):
        row = []
        for j in range(NCH):
            t = em_t[i * 128:(i + 1) * 128, j * CH:(j + 1) * CH]
            mx = t.max()
            mn = t.min()
            if mx == 0.0:
                row.append(SKIP)
            elif mn == 1.0 and mx == 1.0:
                row.append(PLAIN)
            else:
                row.append(MASKED)
        grid.append(tuple(row))
    return tuple(grid)


def _causal_grid():
    g = []
    for i in range(NKS):
        row = []
        for j in range(NCH):
            if i >= 4 * j + 4:
                row.append(SKIP)
            elif i >= 4 * j:
                row.append(MASKED)
            else:
                row.append(PLAIN)
        g.append(tuple(row))
    return tuple(g)


def _is_exact_causal(emaskT_b):
    """True iff exp(mask).T's diagonal band is exactly the causal 0/1
    pattern (off-band is covered by the grid comparison)."""
    p = np.arange(128)[:, None]
    for jj in range(NCH):
        for i in range(4 * jj, 4 * jj + 4):
            t = emaskT_b[i * 128:(i + 1) * 128, jj * CH:(jj + 1) * CH]
            d = i - 4 * jj
            q = np.arange(CH)[None, :]
            want = (p - q + 128 * d <= 0).astype(np.float32)
            if not np.array_equal(t, want):
                return False
    return True


def kernel(hidden_states, cos, sin, attention_mask,
           q_w, k_w, v_w, q_b, k_b, v_b,
           q_A, q_B, k_A, k_B, v_A, v_B, o_w):
    f32 = np.float32
    hidden_states = np.asarray(hidden_states, dtype=f32)
    cos = np.asarray(cos, dtype=f32)
    sin = np.asarray(sin, dtype=f32)
    mask = np.asarray(attention_mask, dtype=f32)[:, 0]  # [B, S, S]

    # host-side shared prep
    with np.errstate(under="ignore", over="ignore"):
        emask = np.exp(np.minimum(mask, 80.0))  # [B, S, S]; clamp avoids inf
    emaskT = [np.ascontiguousarray(emask[b].T) for b in range(B)]
    grids = [_classify(emaskT[b]) for b in range(B)]
    if grids[0] != grids[1]:
        grid = tuple(tuple(MASKED if (grids[0][i][j] != SKIP or grids[1][i][j] != SKIP)
                           else SKIP for j in range(NCH)) for i in range(NKS))
    else:
        grid = grids[0]
    for j in range(NCH):
        if all(grid[i][j] == SKIP for i in range(NKS)):
            grid = tuple(tuple(MASKED for _ in range(NCH)) for _ in range(NKS))
            break

    causal = (grid == _causal_grid()
              and all(_is_exact_causal(emaskT[b]) for b in range(B)))

    nc = _get_program((grid, causal), grid, causal)

    # x_pre[c, p, kt, s'] = x[b][c*CH+s', kt*128+p]
    xT = [np.ascontiguousarray(
        hidden_states[b].reshape(NCH, CH, KT, 128).transpose(0, 3, 2, 1)
        ).astype(NPBF16) for b in range(B)]
    NPF8 = ml_dtypes.float8_e4m3
    xT8 = [np.clip(np.asarray(t, np.float32) * 2.0, -240.0, 240.0).astype(NPF8)
           for t in xT] if causal else None
    cosT = [np.ascontiguousarray(cos[b].T).astype(NPBF16) for b in range(B)]
    ss = np.concatenate([-sin[:, :, :HD // 2], sin[:, :, HD // 2:]], axis=-1)
    ssT = [np.ascontiguousarray(ss[b].T).astype(NPBF16) for b in range(B)]
    emaskT16 = None

    # effective weights: W_eff[outdim, h] = W[outdim, h] + s*(A @ B).T[outdim, h]
    qw_eff = q_w + LORA_SCALE * (q_A @ q_B).T
    kw_eff = k_w + LORA_SCALE * (k_A @ k_B).T
    vw_eff = v_w + LORA_SCALE * (v_A @ v_B).T

    in_maps = []
    for c in range(NCORES):
        b, g = divmod(c, KVH)
        qsl = slice(QD * g, QD * (g + 1))
        ksl = slice(HD * g, HD * (g + 1))
        w_cat = np.concatenate([qw_eff[qsl], kw_eff[ksl], vw_eff[ksl]], axis=0)
        # w_pre[p, t, kt, o] = w_cat[t*128+o, kt*128+p]
        wT_c = w_cat.reshape(NT, 128, KT, 128).transpose(3, 0, 2, 1)
        wT_b = np.ascontiguousarray(wT_c).astype(NPBF16)
        # v bias handled on host: after softmax-normalization its contribution
        # to the output is the constant row o_w @ vb_o (added post-gather)
        bias_cat = np.concatenate([q_b[qsl], k_b[ksl],
                                   np.zeros(HD, f32)]).astype(f32)
        bias_cols = bias_cat.reshape(NT, 128).T  # [128, NT]
        swap_idx = np.concatenate([np.arange(64, 128), np.arange(0, 64)])
        biasT_c = np.ascontiguousarray(
            np.concatenate([bias_cols, bias_cols[swap_idx]], axis=1))  # [128, 2*NT]
        owT_c = o_w[:, qsl].T
        m = {
            "xT": xT[b],
            "wT": wT_b,
            "biasT": biasT_c,
            "cachetag": np.zeros((1, (K_TAG_INT % 97) + 1), f32),
            "cosT": cosT[b],
            "ssT": ssT[b],
            "owT": np.ascontiguousarray(owT_c).astype(NPBF16),
        }
        if causal:
            m["xT8"] = xT8[b]
            # fp8 weights at 32x scale: q0..q3,k straight; v as hi+lo split
            ws = np.asarray(wT_b, np.float32) * 32.0
            qk_hi = np.clip(ws[:, :NT - 1], -240.0, 240.0).astype(NPF8)
            v_hi = np.clip(ws[:, NT - 1:NT], -240.0, 240.0).astype(NPF8)
            v_lo = np.clip(ws[:, NT - 1:NT] - v_hi.astype(np.float32),
                           -240.0, 240.0).astype(NPF8)
            m["w8T"] = np.ascontiguousarray(
                np.concatenate([qk_hi, v_hi, v_lo], axis=1))
        if not causal and any(grid[i][j] == MASKED for i in range(NKS) for j in range(NCH)):
            if emaskT16 is None:
                emaskT16 = [e.astype(NPBF16) for e in emaskT]
            m["emaskT"] = emaskT16[b]
        in_maps.append(m)

    res = run_bass_kernel_spmd(nc, in_maps, core_ids=list(range(NCORES)))
    outs = [np.asarray(r["out_p"], dtype=f32) for r in res.results]
    # v-bias contribution: softmax rows sum to 1, so the +v_b term passes
    # through attention unchanged and adds o_w @ vb_o to every output row
    vb_o = np.empty(NH * HD, f32)
    for g in range(KVH):
        vb_o[QD * g:QD * (g + 1)] = np.tile(v_b[HD * g:HD * (g + 1)], GQ)
    delta = (o_w.astype(f32) @ vb_o)[None, :]  # [1, H]
    full = np.empty((B, S, H), f32)
    for b in range(B):
        full[b] = outs[KVH * b]
        for g in range(1, KVH):
            full[b] += outs[KVH * b + g]
        full[b] += delta
    return full



# revision 75
# speedup vs baseline: 1.0139x; 1.0139x over previous
"""Trainium2 Bass kernel for LoRA-fused QKV + RoPE + GQA causal attention + o_proj.

Problem (hardcoded): B=2, S=2048, H=2048, NH=16, KVH=4, HD=128, R=16.

Sharding: 8 cores = batch(2) x kv-head-group(4). Core c handles batch b=c//4,
kv head g=c%4 (q heads 4g..4g+3). Each core computes its 4 heads' attention and
a partial o_proj ([S,H] partial over its 512 o-dims); host sums 4 partials per
batch.

v3 design (fp8-DoubleRow mixed precision over the bf16 v2):
- Transposed space throughout: projections produce qT/kT/vT [d, s], scoresT
  [ks, qs] feeds AV directly, o_proj consumes outT [d, s] stationary. PSUM
  accumulation is always fp32; LoRA folded into W on the host; biases applied
  during PSUM evacuation / rope.
- fp8 (e4m3) DoubleRow matmuls — contracting ks/kt tile PAIRS (256-deep) at
  ~1.5x the bf16 pair rate — for: AV + the all-ones denominator matmul on
  causal chunks 1..3 (attn weights written by exp directly in fp8 with a -3
  shift that cancels in the softmax ratio; v also fp8), and the chunk 1..3
  projections (q/k straight fp8 at x*2 / w*32, dequant 1/64 in the evac Copy
  scale; v as an fp8 hi+lo split-W pair which restores ~bf16 precision).
- Chunk 0 (rows/keys 0..511) stays entirely bf16: early causal rows attend
  few keys, so quantization noise there does not average out. QK and o_proj
  stay bf16 everywhere (128-deep / precision-critical). Measured rel err
  6.5e-3 vs the 2e-2 gate.
- Softmax: no max-subtraction; one ACTIVATE per score-tile pair; denominator
  via ones-stationary matmul (broadcasts across partitions); normalization on
  DVE with reciprocal_approx_fast.
- Scheduling: proj chunks in order [1,2,0,3] so the PE warms up on the small
  fp8 startup set (w8+x8 ~2.9MB) while chunk 0's 5.25MB bf16 set streams
  behind it (early DMA is ramp-limited to ~3MB/20us). o_proj(prev) units are
  interleaved INTO the scalar-bound attention phase (PE chews o_proj matmuls
  while waiting on exp); attention itself is pipelined 2 score-pairs deep.
  Output tiles stream back round-robin on the sync/gpsimd queues (scalar kept
  free for exp during attention).
"""

import hashlib
import numpy as np
import ml_dtypes

import concourse.bass as bass
import concourse.mybir as mybir
import concourse.tile as tile
from concourse import bacc
from concourse.bass_utils import run_bass_kernel_spmd

B, S, H = 2, 2048, 2048
NH, KVH, HD = 16, 4, 128
R = 16
LORA_SCALE = 32.0 / 16.0
ATTN_SCALE = HD ** -0.5

NCORES = 8
GQ = NH // KVH          # 4 q heads per core
NT = GQ + 2             # 6 projection tiles: 4 q heads, 1 k, 1 v
QD = GQ * HD            # 512
CH = 512                # s-chunk width
NCH = S // CH           # 4 s-chunks
KT = H // 128           # 16 contraction k-tiles
NKS = S // 128          # 16 ks tiles
F32 = mybir.dt.float32
F32R = mybir.dt.float32r
BF16 = mybir.dt.bfloat16
F8 = mybir.dt.float8e4
NPBF16 = ml_dtypes.bfloat16
# fp8 attention-weight path (causal chunks j>=1): exp is shifted by EXP_SHIFT
# so e^(s*scale+shift) stays under the e4m3 max (240); the shift scales the
# AV numerator and the ones-denominator identically, so it cancels exactly.
EXP_SHIFT = -3.0

# tile classification codes (host-computed from exp(mask) tiles)
SKIP, PLAIN, MASKED = 0, 1, 2

# content tag: force a fresh NEFF cache key whenever this file changes
with open(__file__, "rb") as _f:
    KTAG = hashlib.sha1(_f.read()).hexdigest()[:10]
K_TAG_INT = int(KTAG, 16)


def _build(cls_grid, causal):
    """Build the SPMD program. cls_grid[i][j] in {SKIP, PLAIN, MASKED} for
    scoresT tile (ks_tile i, qs_chunk j). causal=True generates the diagonal
    mask tiles on device (no emaskT input)."""
    nc = bacc.Bacc("TRN2", target_bir_lowering=False)

    # host-packed for contiguous per-partition DMA:
    # x_pre[c, p, kt, s'] = x[b][s = c*CH+s', h = kt*128+p]  (bf16)
    xT = nc.dram_tensor("xT", [NCH, 128, KT, CH], BF16, kind="ExternalInput")
    # w_pre[p, t, kt, o] = w_eff[h = kt*128+p, t*128+o]  (bf16, LoRA folded)
    wT = nc.dram_tensor("wT", [128, NT, KT, 128], BF16, kind="ExternalInput")
    # fp8 copies for the DoubleRow q/k projections of chunks 1..3:
    # xT8 = e4m3(2*x), w8T = e4m3(32*w_eff[q0..q3,k]); dequant 1/64 on evac
    xT8 = w8T = None
    if causal:
        xT8 = nc.dram_tensor("xT8", [NCH, 128, KT, CH], F8, kind="ExternalInput")
        # 7 t-slices: q0..q3, k (straight) + v_hi, v_lo (split residual pair)
        w8T = nc.dram_tensor("w8T", [128, NT + 1, KT, 128], F8,
                             kind="ExternalInput")
    # [:, 0:NT] plain bias columns; [:, NT:2*NT] partition-swapped (rotate-half)
    biasT = nc.dram_tensor("biasT", [128, 2 * NT], F32, kind="ExternalInput")
    # cache-buster: the PJRT NEFF cache hashes the HLO minus backend_config
    DL = (K_TAG_INT % 97) + 1
    dummy = nc.dram_tensor("cachetag", [1, DL], F32, kind="ExternalInput")
    cosT = nc.dram_tensor("cosT", [HD, S], BF16, kind="ExternalInput")
    ssT = nc.dram_tensor("ssT", [HD, S], BF16, kind="ExternalInput")
    any_masked = any(cls_grid[i][j] == MASKED for i in range(NKS) for j in range(NCH))
    emaskT = None
    if not causal and any_masked:
        emaskT = nc.dram_tensor("emaskT", [S, S], BF16, kind="ExternalInput")
    owT = nc.dram_tensor("owT", [QD, H], BF16, kind="ExternalInput")
    out_p = nc.dram_tensor("out_p", [S, H], BF16, kind="ExternalOutput")

    live_per_j = [[i for i in range(NKS) if cls_grid[i][jj] != SKIP]
                  for jj in range(NCH)]
    masked_per_j = [[i for i in range(NKS) if cls_grid[i][jj] == MASKED]
                    for jj in range(NCH)]
    need = [max(jj, max(live_per_j[jj]) // (CH // 128)) for jj in range(NCH)]
    QCH_BUFS = max(2, max(need[jj] - jj for jj in range(NCH)) + 1)
    if causal:
        QCH_BUFS = 3   # proj order [1,2,0,3]: three q chunks live at once

    with tile.TileContext(nc) as tc:
        from concourse.masks import make_identity
        with tc.tile_pool(name="consts", bufs=1) as consts, \
             tc.tile_pool(name="persist", bufs=1) as persist, \
             tc.tile_pool(name="qch", bufs=QCH_BUFS) as qch_pool, \
             tc.tile_pool(name="outp", bufs=2) as outp_pool, \
             tc.tile_pool(name="p1", bufs=5) as p1, \
             tc.tile_pool(name="xch", bufs=2) as xch_pool, \
             tc.tile_pool(name="att", bufs=5) as att_pool, \
             tc.tile_pool(name="stgp", bufs=8) as stgp, \
             tc.tile_pool(name="fin", bufs=2) as fin, \
             tc.tile_pool(name="pp_pair", bufs=2, space="PSUM") as pp_pair, \
             tc.tile_pool(name="pp_o", bufs=2, space="PSUM") as pp_o, \
             tc.tile_pool(name="pp_sum", bufs=1, space="PSUM") as pp_sum, \
             tc.tile_pool(name="pp_t", bufs=1, space="PSUM") as pp_t:

            # causal proj order [1, 2, 0, 3]: the first two chunks run on the
            # small fp8 weight/activation set (w8 1.8MB + x8 1MB each) so the
            # PE starts ~4x sooner than the 5.25MB bf16 chunk-0 set allows;
            # chunk 0's bf16 weights stream in behind them.
            proj_order = [1, 2, 0, 3] if causal else list(range(NCH))

            # gpsimd-generated const scratch emitted BEFORE any DMA
            # descriptor hits the gpsimd queue: the queue stalls ~20us on
            # DMA credits, which otherwise gates permb (rope stationary,
            # needed ~13us) and identb (v transpose, needed ~21us)
            ident_f = consts.tile([128, 128], F32, tag="ident_f")
            nc.gpsimd.memset(ident_f, 0.0)
            nc.gpsimd.affine_select(
                out=ident_f, in_=ident_f,
                compare_op=mybir.AluOpType.not_equal,
                fill=1.0, base=0, channel_multiplier=1, pattern=[[-1, 128]],
            )
            # half-rotation permutation: perm[p, q] = 1 iff q == (p+64)%128.
            # Used as a matmul stationary to compute rotate-half on the PE.
            perm_f = consts.tile([128, 128], F32, tag="perm_f")
            nc.gpsimd.memset(perm_f, 0.0)
            for base in (64, -64):
                nc.gpsimd.affine_select(
                    out=perm_f, in_=perm_f,
                    compare_op=mybir.AluOpType.not_equal,
                    fill=1.0, base=base, channel_multiplier=1,
                    pattern=[[-1, 128]],
                )

            x_tiles = {}
            x8_tiles = {}

            def emit_x_dma(c, fine=False):
                if causal and c >= 1:
                    # chunks >=1 project entirely from x8 (v uses split-W fp8)
                    x8_c = xch_pool.tile([128, KT, CH], F8, tag="x8_c",
                                         bufs=3, name=f"x8_{c}")
                    x8_tiles[c] = x8_c
                    if fine:
                        for kp in range(KT // 2):
                            q = nc.sync if kp % 2 == 0 else nc.gpsimd
                            q.dma_start(out=x8_c[:, bass.ds(kp * 2, 2), :],
                                        in_=xT8[c, :, bass.ds(kp * 2, 2), :])
                    else:
                        nc.sync.dma_start(out=x8_c[:, bass.ds(0, 8), :],
                                          in_=xT8[c, :, bass.ds(0, 8), :])
                        nc.gpsimd.dma_start(out=x8_c[:, bass.ds(8, 8), :],
                                            in_=xT8[c, :, bass.ds(8, 8), :])
                    return
                bufs = 1 if causal else 2
                x_c = xch_pool.tile([128, KT, CH], BF16, tag="x_c",
                                    bufs=bufs, name=f"x_{c}")
                x_tiles[c] = x_c
                if fine:
                    # 16 single-kt pieces: first matmul starts after 0.13MB
                    for kt in range(KT):
                        q = nc.sync if kt % 2 == 0 else nc.gpsimd
                        q.dma_start(out=x_c[:, bass.ds(kt, 1), :],
                                    in_=xT[c, :, bass.ds(kt, 1), :])
                else:
                    qs = [nc.sync, nc.gpsimd, nc.sync, nc.gpsimd]
                    for kq in range(4):
                        qs[kq].dma_start(out=x_c[:, bass.ds(kq * 4, 4), :],
                                         in_=xT[c, :, bass.ds(kq * 4, 4), :])

            cs_tiles = {}

            def emit_cs_dma(c, q=None):
                q = q or nc.sync
                sl = bass.ds(c * CH, CH)
                cos_c = xch_pool.tile([128, CH], BF16, tag="cos_c", bufs=3,
                                      name=f"cos_{c}")
                q.dma_start(out=cos_c, in_=cosT[:, sl])
                ss_c = xch_pool.tile([128, CH], BF16, tag="ss_c", bufs=3,
                                     name=f"ss_{c}")
                q.dma_start(out=ss_c, in_=ssT[:, sl])
                cs_tiles[c] = (cos_c, ss_c)

            # ---- startup DMAs ----
            w_sb = persist.tile([128, NT, KT, 128], BF16, tag="w_sb")
            w8_sb = (persist.tile([128, NT + 1, KT, 128], F8, tag="w8_sb",
                                  name="w8_sb") if causal else None)
            ow_sb = persist.tile([128, GQ, H], BF16, tag="ow_sb")
            ow_done = [False]
            if causal:
                # critical path: w8 (t-need order, t0 split fine) on scalar;
                # x8(1) fine on sync/gpsimd
                for hp in range(2):
                    nc.scalar.dma_start(out=w8_sb[:, 0, bass.ds(hp * 8, 8), :],
                                        in_=w8T[:, 0, bass.ds(hp * 8, 8), :])
                emit_x_dma(proj_order[0], fine=True)
                for t in [1, GQ, 2, 3]:
                    nc.scalar.dma_start(out=w8_sb[:, t, :, :],
                                        in_=w8T[:, t, :, :])
                nc.scalar.dma_start(out=w8_sb[:, bass.ds(NT - 1, 2), :, :],
                                    in_=w8T[:, bass.ds(NT - 1, 2), :, :])
                bias_sb = consts.tile([128, 2 * NT], F32, tag="bias_sb")
                nc.gpsimd.dma_start(out=bias_sb, in_=biasT[:, :])
                emit_cs_dma(proj_order[0], q=nc.gpsimd)
                # second fp8 chunk + its rope tables
                emit_x_dma(proj_order[1])
                emit_cs_dma(proj_order[1], q=nc.gpsimd)
                # chunk 0's bf16 set streams in behind (t-need order). x0
                # goes AHEAD of the later w tiles on sync/gpsimd: proj(0)'s
                # first t-group contracts over the WHOLE x0 chunk, while
                # w GQ/2/3/v aren't consumed until later t-groups
                for kq in range(4):
                    nc.scalar.dma_start(out=w_sb[:, 0, bass.ds(kq * 4, 4), :],
                                        in_=wT[:, 0, bass.ds(kq * 4, 4), :])
                nc.scalar.dma_start(out=w_sb[:, 1, :, :], in_=wT[:, 1, :, :])
                emit_x_dma(0)
                nc.sync.dma_start(out=w_sb[:, GQ, :, :], in_=wT[:, GQ, :, :])
                nc.gpsimd.dma_start(out=w_sb[:, 2, :, :], in_=wT[:, 2, :, :])
                nc.sync.dma_start(out=w_sb[:, 3, :, :], in_=wT[:, 3, :, :])
                nc.gpsimd.dma_start(out=w_sb[:, NT - 1, :, :],
                                    in_=wT[:, NT - 1, :, :])
                emit_cs_dma(0)
            else:
                # t0 in 4 fine pieces so the first matmul starts after ~0.16MB
                for kq in range(4):
                    nc.scalar.dma_start(out=w_sb[:, 0, bass.ds(kq * 4, 4), :],
                                        in_=wT[:, 0, bass.ds(kq * 4, 4), :])
                emit_x_dma(0, fine=True)
                for t in [1, GQ, 2]:   # proj t-need order
                    nc.scalar.dma_start(out=w_sb[:, t, :, :], in_=wT[:, t, :, :])
                nc.sync.dma_start(out=w_sb[:, 3, :, :], in_=wT[:, 3, :, :])
                nc.gpsimd.dma_start(out=w_sb[:, NT - 1, :, :],
                                    in_=wT[:, NT - 1, :, :])
                emit_cs_dma(0)
                bias_sb = consts.tile([128, 2 * NT], F32, tag="bias_sb")
                nc.gpsimd.dma_start(out=bias_sb, in_=biasT[:, :])
            dummy_sb = consts.tile([1, 128], F32, tag="dummy_sb")
            nc.gpsimd.dma_start(out=dummy_sb[:, 0:DL], in_=dummy[:, :])

            # ---- small constants ----
            # full 128-col all-ones stationary: the denominator matmul then
            # broadcasts the column sums across all 128 PSUM partitions (no
            # gpsimd partition_broadcast needed) and keeps LDWEIGHTS
            # pull-ahead working (no col_grp restriction)
            ones_mat = consts.tile([128, 128], BF16, tag="ones_mat")
            nc.vector.memset(ones_mat, 1.0)
            # fp8 all-ones stationary PAIR for DoubleRow denominator matmuls
            ones8 = consts.tile([128, 2, 128], F8, tag="ones8")
            nc.vector.memset(ones8, 1.0)
            # per-partition bias column holding EXP_SHIFT for the fp8 exp
            eshift = consts.tile([128, 1], F32, tag="eshift")
            nc.vector.memset(eshift, EXP_SHIFT)
            identb = consts.tile([128, 128], BF16, tag="identb")
            nc.vector.tensor_copy(out=identb, in_=ident_f)
            permb = consts.tile([128, 128], BF16, tag="permb")
            nc.vector.tensor_copy(out=permb, in_=perm_f)

            # causal: single [128,128] lower-triangle mask; diag tiles are
            # processed as singles restricted to their live column span
            # [128*d, CH), where only the first 128 columns are triangular
            tri = None
            if causal:
                scratch = consts.tile([128, 128], F32, tag="tri_scratch")
                nc.gpsimd.memset(scratch, 0.0)
                nc.gpsimd.affine_select(
                    out=scratch, in_=scratch,
                    compare_op=mybir.AluOpType.is_gt,
                    fill=1.0,
                    base=0,
                    channel_multiplier=1,
                    pattern=[[-1, 128]],
                )
                tri = consts.tile([128, 128], BF16, tag="tri")
                nc.vector.tensor_copy(out=tri, in_=scratch)

            # not-yet-fetched chunks are prefetched lazily (first attention)
            # so they don't steal startup fabric bandwidth
            n_startup = 3 if causal else 1
            to_fetch = [c for c in proj_order[n_startup:]]

            def emit_prefetch():
                if to_fetch:
                    c = to_fetch.pop(0)
                    emit_x_dma(c)
                    emit_cs_dma(c)
                if not ow_done[0]:
                    ow_done[0] = True
                    nc.gpsimd.dma_start(
                        out=ow_sb, in_=owT.rearrange("(g p) n -> p g n", p=128))

            # ---- persistent tiles ----
            kT_full = persist.tile([128, S], BF16, tag="kT_full")
            v_nat = persist.tile([128, NKS, 128], BF16, tag="v_nat")  # [ks, tile, d]
            v_nat8 = (persist.tile([128, NKS, 128], F8, tag="v_nat8",
                                   name="v_nat8") if causal else None)

            out_dma_q = [nc.sync, nc.gpsimd]
            out_dma_n = [0]

            def oproj_units(args, final=False, during_attn=False):
                """One unit per [128,CH] output tile: 4 accum matmuls + evac +
                DMA. during_attn keeps the scalar engine free for exp."""
                cc, outT_ch = args
                if final:
                    qlist = [nc.sync, nc.gpsimd, nc.scalar]
                elif during_attn:
                    # keep the scalar queue free for exp during attention
                    qlist = [nc.sync, nc.gpsimd]
                else:
                    qlist = [nc.gpsimd, nc.scalar]

                def unit(st4, nch):
                    ssl = bass.ds(st4 * 128, 128)
                    dsl = bass.ds((cc * (CH // 128) + st4) * 128, 128)
                    pop_deferred_v()
                    nsl = bass.ds(nch * CH, CH)
                    ps3 = pp_pair.tile([128, 2, CH], F32, tag="pair", name="ps3")
                    g = st4 * NCH + nch
                    half = g % 2
                    for h in range(GQ):
                        nc.tensor.matmul(ps3[:, half, :], outT_ch[h][:, ssl],
                                         ow_sb[:, h, nsl],
                                         start=(h == 0), stop=(h == GQ - 1))
                    stg = stgp.tile([128, CH], BF16, tag="stg")
                    if during_attn or g % 2 == 0:
                        nc.vector.tensor_copy(out=stg, in_=ps3[:, half, :])
                    else:
                        nc.scalar.activation(out=stg, in_=ps3[:, half, :],
                                             func=mybir.ActivationFunctionType.Copy)
                    q = qlist[out_dma_n[0] % len(qlist)]
                    out_dma_n[0] += 1
                    q.dma_start(out=out_p[dsl, nsl], in_=stg)

                return [(lambda st4=st4, nch=nch: unit(st4, nch))
                        for st4 in range(CH // 128) for nch in range(NCH)]

            def emit_oproj(args, final=False):
                for u in oproj_units(args, final=final):
                    u()

            q_chunks = {}
            deferred_v = []

            def pop_deferred_v():
                """Emit ONE pending v transpose (or nothing)."""
                if not deferred_v:
                    return
                c, vT_c, i4 = deferred_v.pop(0)
                i = c * (CH // 128) + i4
                ps_t = pp_t.tile([128, 128], BF16, tag="ps_t", name="ps_t")
                nc.tensor.transpose(ps_t, vT_c[:, bass.ds(i4 * 128, 128)], identb)
                nc.vector.tensor_copy(out=v_nat[:, i, :], in_=ps_t)
                if v_nat8 is not None:
                    nc.vector.tensor_copy(out=v_nat8[:, i, :], in_=ps_t)

            def flush_deferred_v():
                while deferred_v:
                    pop_deferred_v()

            def emit_proj(c, evac_vec=False):
                x_c = x_tiles.get(c)
                cos_c, ss_c = cs_tiles[c]
                sl = bass.ds(c * CH, CH)

                q_ch = [qch_pool.tile([128, CH], BF16, tag=f"qch{h}", name=f"qch{h}_{c}")
                        for h in range(GQ)]
                q_chunks[c] = q_ch

                pending_rope = []

                def flush_rope():
                    # rotate-half via PE permutation matmul, then rope on DVE:
                    # dst = (raw+b)*cos + (perm@raw + swap(b))*ss
                    for t, raw in pending_rope:
                        bias_col = bias_sb[:, t:t + 1]
                        bias_sw = bias_sb[:, NT + t:NT + t + 1]
                        ps_sw = pp_t.tile([128, CH], F32, tag="ps_t", name="ps_sw")
                        nc.tensor.matmul(ps_sw, permb, raw, start=True, stop=True)
                        sw = p1.tile([128, CH], BF16, tag="sw")
                        nc.vector.scalar_tensor_tensor(
                            out=sw, in0=ps_sw, scalar=bias_sw, in1=ss_c,
                            op0=mybir.AluOpType.add, op1=mybir.AluOpType.mult)
                        dst = q_ch[t] if t < GQ else kT_full[:, sl]
                        nc.vector.scalar_tensor_tensor(
                            out=dst, in0=raw, scalar=bias_col, in1=cos_c,
                            op0=mybir.AluOpType.add, op1=mybir.AluOpType.mult)
                        nc.vector.tensor_add(dst, dst, sw)
                    pending_rope.clear()

                def evac_copy(dst, src, dq):
                    # evac_vec: during the startup phase the scalar queue is
                    # blocked on DMA-descriptor credit stalls — evacuate on
                    # the vector engine so the PE pipeline isn't gated on it
                    if evac_vec:
                        if dq:
                            nc.vector.tensor_scalar_mul(dst, src, 1.0 / 64.0)
                        else:
                            nc.vector.tensor_copy(out=dst, in_=src)
                    else:
                        nc.scalar.activation(out=dst, in_=src,
                                             func=mybir.ActivationFunctionType.Copy,
                                             scale=(1.0 / 64.0) if dq else 1.0)

                def evac_tile(t, ps, half, dq=False):
                    if t == NT - 1:   # v (bias folded on host); transpose deferred
                        vT_c = p1.tile([128, CH], BF16, tag="vT_c")
                        evac_copy(vT_c, ps[:, half, :], dq)
                        for i4 in range(CH // 128):
                            deferred_v.append((c, vT_c, i4))
                    else:
                        raw = p1.tile([128, CH], BF16, tag="raw")
                        evac_copy(raw, ps[:, half, :], dq)
                        pending_rope.append((t, raw))

                # fp8 DoubleRow projections for chunks >= 1: q/k use straight
                # fp8 weights (their quantization noise averages out through
                # softmax over >=512 keys); v uses a hi+lo split-W pair, which
                # restores ~bf16 weight precision at the same PE cost
                fp8p = w8_sb is not None and c >= 1

                # t order: q0, q1, k, q2, q3, v; each tile's rope flushes one
                # MM-group later so the PSUM evacuation copy is never waited on
                for n, t in enumerate([0, 1, GQ, 2, 3, NT - 1]):
                    ps = pp_pair.tile([128, 2, CH], F32, tag="pair",
                                      name="ps_p")
                    half = n % 2
                    if fp8p:
                        x8_c = x8_tiles[c]
                        slices = [t] if t != NT - 1 else [NT - 1, NT]
                        total = len(slices) * (KT // 2)
                        nmm = 0
                        for w8t in slices:
                            for kp in range(KT // 2):
                                nc.tensor.matmul(
                                    ps[:, half, :],
                                    w8_sb[:, w8t, bass.ds(2 * kp, 2), :],
                                    x8_c[:, bass.ds(2 * kp, 2), :],
                                    start=(nmm == 0), stop=(nmm == total - 1),
                                    perf_mode=mybir.MatmulPerfMode.DoubleRow)
                                nmm += 1
                    else:
                        for kt in range(KT):
                            nc.tensor.matmul(ps[:, half, :], w_sb[:, t, kt, :],
                                             x_c[:, kt, :],
                                             start=(kt == 0), stop=(kt == KT - 1))
                    flush_rope()
                    pop_deferred_v()
                    evac_tile(t, ps, half, dq=fp8p)
                flush_rope()

            def emit_attention(j, bg=()):
                """Emits the attention for chunk j with a 2-pair software
                pipeline across head boundaries. bg: list of o_proj unit
                emitters interleaved into the (scalar-bound) attention phase
                so the PE chews o_proj matmuls while waiting on exp."""
                flush_deferred_v()
                sl = bass.ds(j * CH, CH)
                live = live_per_j[j]
                masked = set(masked_per_j[j])
                # entry list: (i0, i1 or None, qlo, diag). Causal: plain tiles
                # paired full-width; diag tiles as singles restricted to their
                # live span [qlo, CH) with a [128,128] triangle mask on the
                # first 128 live columns.
                ents = []   # (i0, i1|None, s0, s1|None, diag)
                if causal:
                    plain = [i for i in live if i not in masked]
                    for n in range(0, len(plain), 2):
                        i1 = plain[n + 1] if n + 1 < len(plain) else None
                        ents.append((plain[n], i1, 0, 0 if i1 is not None else None,
                                     False))
                    dg = sorted(masked)
                    for n in range(0, len(dg), 2):
                        i0 = dg[n]
                        i1 = dg[n + 1] if n + 1 < len(dg) else None
                        s0 = 128 * (i0 - 4 * j)
                        s1 = 128 * (i1 - 4 * j) if i1 is not None else None
                        ents.append((i0, i1, s0, s1, True))
                else:
                    for n in range(0, len(live), 2):
                        i1 = live[n + 1] if n + 1 < len(live) else None
                        ents.append((live[n], i1, 0, 0 if i1 is not None else None,
                                     False))

                outT_ch = [outp_pool.tile([128, CH], BF16, tag=f"outT{h}",
                                          name=f"outT{h}_{j}") for h in range(GQ)]
                q_ch = q_chunks[j]

                # global pipeline over (head, entry)
                work = [(h, e) for h in range(GQ) for e in ents]
                NP = len(work)
                attns = {}

                # fp8 DoubleRow path: causal chunks j>=1 (every row there has
                # >=512 live keys, so fp8 quantization noise on the attention
                # weights and v averages out; chunk 0 stays bf16)
                fp8 = causal and j >= 1

                def emit_qk_exp(n):
                    h, (i0, i1, s0, s1, diag) = work[n]
                    qh = q_ch[h]
                    sp0 = bass.ds(s0, CH - s0)
                    pr = pp_pair.tile([128, 2, CH], F32, tag="pair", name="pr")
                    nc.tensor.matmul(pr[:, 0, sp0],
                                     kT_full[:, bass.ds(i0 * 128, 128)],
                                     qh[:, sp0], start=True, stop=True)
                    if i1 is not None:
                        sp1 = bass.ds(s1, CH - s1)
                        nc.tensor.matmul(pr[:, 1, sp1],
                                         kT_full[:, bass.ds(i1 * 128, 128)],
                                         qh[:, sp1], start=True, stop=True)
                    if fp8:
                        attn = att_pool.tile([128, 2, CH], F8, tag="attn8")
                    else:
                        attn = att_pool.tile([128, 2, CH], BF16, tag="attn")
                    # one rectangular exp over both halves from s0 (s0 <= s1;
                    # half1's [s0,s1) region is garbage: never read in the
                    # bf16 path, zeroed below before the DoubleRow matmul)
                    src = pr[:, :, sp0] if i1 is not None else pr[:, 0:1, sp0]
                    dst = attn[:, :, sp0] if i1 is not None else attn[:, 0:1, sp0]
                    nc.scalar.activation(out=dst, in_=src,
                                         func=mybir.ActivationFunctionType.Exp,
                                         scale=float(ATTN_SCALE),
                                         bias=eshift[:, :] if fp8 else 0.0)
                    if diag:
                        nc.vector.tensor_mul(attn[:, 0, bass.ds(s0, 128)],
                                             attn[:, 0, bass.ds(s0, 128)], tri)
                        if i1 is not None:
                            nc.vector.tensor_mul(attn[:, 1, bass.ds(s1, 128)],
                                                 attn[:, 1, bass.ds(s1, 128)], tri)
                            if fp8 and s1 > s0:
                                # zero half1's dead span so DoubleRow can feed
                                # both halves over the shared [s0, CH) span
                                nc.vector.memset(
                                    attn[:, 1, bass.ds(s0, s1 - s0)], 0.0)
                    elif (i0 in masked) or (i1 in masked):
                        mt = att_pool.tile([128, 2, CH], BF16, tag="m_tile",
                                           bufs=3, name=f"mt_{j}_{h}_{i0}")
                        nc.gpsimd.dma_start(out=mt[:, 0, :],
                                            in_=emaskT[bass.ds(i0 * 128, 128), sl])
                        if i1 is not None:
                            nc.gpsimd.dma_start(
                                out=mt[:, 1, :],
                                in_=emaskT[bass.ds(i1 * 128, 128), sl])
                        msl = attn if i1 is not None else attn[:, 0:1, :]
                        mm = mt if i1 is not None else mt[:, 0:1, :]
                        nc.vector.tensor_mul(msl, msl, mm)
                    attns[n] = attn

                def emit_av(n):
                    h, (i0, i1, s0, s1, diag) = work[n]
                    attn = attns.pop(n)
                    sp0 = bass.ds(s0, CH - s0)
                    pidx = n % len(ents)
                    first, last = (pidx == 0), (pidx == len(ents) - 1)
                    ps_o = st_o[h]
                    ps_sum = st_sum[h]
                    if fp8 and i1 is not None:
                        # one DoubleRow matmul contracts both ks tiles of the
                        # pair (i1 == i0+1 so the v pair is contiguous)
                        assert i1 == i0 + 1
                        nc.tensor.matmul(ps_o[:, sp0],
                                         v_nat8[:, bass.ds(i0, 2), :],
                                         attn[:, :, sp0],
                                         start=first, stop=last,
                                         perf_mode=mybir.MatmulPerfMode.DoubleRow)
                        nc.tensor.matmul(ps_sum[:, sp0], ones8,
                                         attn[:, :, sp0],
                                         start=first, stop=last,
                                         perf_mode=mybir.MatmulPerfMode.DoubleRow)
                        if last:
                            finalize(h)
                        return
                    v0 = v_nat8[:, i0, :] if fp8 else v_nat[:, i0, :]
                    o0 = ones8[:, 0, :] if fp8 else ones_mat
                    nc.tensor.matmul(ps_o[:, sp0], v0,
                                     attn[:, 0, sp0],
                                     start=first, stop=(last and i1 is None))
                    nc.tensor.matmul(ps_sum[:, sp0], o0,
                                     attn[:, 0, sp0],
                                     start=first, stop=(last and i1 is None))
                    if i1 is not None:
                        sp1 = bass.ds(s1, CH - s1)
                        nc.tensor.matmul(ps_o[:, sp1], v_nat[:, i1, :],
                                         attn[:, 1, sp1],
                                         start=False, stop=last)
                        nc.tensor.matmul(ps_sum[:, sp1], ones_mat,
                                         attn[:, 1, sp1],
                                         start=False, stop=last)
                    if last:
                        finalize(h)

                st_o, st_sum = {}, {}

                def start_head(h):
                    st_o[h] = pp_o.tile([128, CH], F32, tag="ps_o", name="ps_o")
                    st_sum[h] = pp_sum.tile([128, CH], F32, tag="ps_sum",
                                            name="ps_sum")

                def finalize(h):
                    # ps_sum already holds the denominator in every partition
                    recip = fin.tile([128, CH], F32, tag="recip")
                    nc.vector.reciprocal_approx_fast(out=recip, in_=st_sum[h])
                    nc.vector.tensor_mul(outT_ch[h], st_o[h], recip)

                PIPE = 4
                npairs = len(ents)
                bg = list(bg)
                bgi = [0]

                def pump_bg():
                    if bgi[0] < len(bg):
                        bg[bgi[0]]()
                        bgi[0] += 1

                every = max(1, NP // len(bg)) if bg else 0
                for n in range(NP):
                    if n % npairs == 0:
                        start_head(work[n][0])
                    emit_qk_exp(n)
                    # bg unit BEFORE the AV: the PE queue is in-order, so the
                    # o_proj matmuls fill the wait for exp(n-PIPE) instead of
                    # sitting blocked behind the stalled AV
                    if bg and n >= PIPE and n % every == 0:
                        pump_bg()
                    if n >= PIPE:
                        emit_av(n - PIPE)
                for n in range(max(0, NP - PIPE), NP):
                    emit_av(n)
                while bgi[0] < len(bg):
                    pump_bg()
                return outT_ch

            # ---- main fused loop: proj chunks in proj_order; attention(j)
            # as soon as chunks 0..need[j] are projected, with the previous
            # o_proj's units interleaved into the scalar-bound attention ----
            pending_oproj = None
            done_proj = set()
            attended = set()
            for pi, p in enumerate(proj_order):
                emit_proj(p, evac_vec=causal and pi < 2)
                done_proj.add(p)
                first_att = True
                for j in range(NCH):
                    if j in attended:
                        continue
                    if all(k in done_proj for k in range(need[j] + 1)):
                        if first_att:
                            # prefetch next chunk's inputs during attention
                            emit_prefetch()
                            first_att = False
                        bg = (oproj_units(pending_oproj, during_attn=True)
                              if pending_oproj is not None else [])
                        pending_oproj = (j, emit_attention(j, bg))
                        attended.add(j)
                if first_att:
                    emit_prefetch()
                    if pending_oproj is not None:
                        emit_oproj(pending_oproj)
                        pending_oproj = None

            if pending_oproj is not None:
                emit_oproj(pending_oproj, final=True)
                pending_oproj = None

    nc.finalize()
    return nc


_cache = {}


def _get_program(key, cls_grid, causal):
    if key not in _cache:
        _cache[key] = _build(cls_grid, causal)
    return _cache[key]


def _classify(em_t):
    """em_t: exp(mask).T [S, S] (ks, qs). Returns tuple-of-tuples class grid
    [NKS][NCH]."""
    grid = []
    for i in range(NKS):
        row = []
        for j in range(NCH):
            t = em_t[i * 128:(i + 1) * 128, j * CH:(j + 1) * CH]
            mx = t.max()
            mn = t.min()
            if mx == 0.0:
                row.append(SKIP)
            elif mn == 1.0 and mx == 1.0:
                row.append(PLAIN)
            else:
                row.append(MASKED)
        grid.append(tuple(row))
    return tuple(grid)


def _causal_grid():
    g = []
    for i in range(NKS):
        row = []
        for j in range(NCH):
            if i >= 4 * j + 4:
                row.append(SKIP)
            elif i >= 4 * j:
                row.append(MASKED)
            else:
                row.append(PLAIN)
        g.append(tuple(row))
    return tuple(g)


def _is_exact_causal(emaskT_b):
    """True iff exp(mask).T's diagonal band is exactly the causal 0/1
    pattern (off-band is covered by the grid comparison)."""
    p = np.arange(128)[:, None]
    for jj in range(NCH):
        for i in range(4 * jj, 4 * jj + 4):
            t = emaskT_b[i * 128:(i + 1) * 128, jj * CH:(jj + 1) * CH]
            d = i - 4 * jj
            q = np.arange(CH)[None, :]
            want = (p - q + 128 * d <= 0).astype(np.float32)
            if not np.array_equal(t, want):
                return False
    return True


def kernel(hidden_states, cos, sin, attention_mask,
           q_w, k_w, v_w, q_b, k_b, v_b,
           q_A, q_B, k_A, k_B, v_A, v_B, o_w):
    f32 = np.float32
    hidden_states = np.asarray(hidden_states, dtype=f32)
    cos = np.asarray(cos, dtype=f32)
    sin = np.asarray(sin, dtype=f32)
    mask = np.asarray(attention_mask, dtype=f32)[:, 0]  # [B, S, S]

    # host-side shared prep
    with np.errstate(under="ignore", over="ignore"):
        emask = np.exp(np.minimum(mask, 80.0))  # [B, S, S]; clamp avoids inf
    emaskT = [np.ascontiguousarray(emask[b].T) for b in range(B)]
    grids = [_classify(emaskT[b]) for b in range(B)]
    if grids[0] != grids[1]:
        grid = tuple(tuple(MASKED if (grids[0][i][j] != SKIP or grids[1][i][j] != SKIP)
                           else SKIP for j in range(NCH)) for i in range(NKS))
    else:
        grid = grids[0]
    for j in range(NCH):
        if all(grid[i][j] == SKIP for i in range(NKS)):
            grid = tuple(tuple(MASKED for _ in range(NCH)) for _ in range(NKS))
            break

    causal = (grid == _causal_grid()
              and all(_is_exact_causal(emaskT[b]) for b in range(B)))

    nc = _get_program((grid, causal), grid, causal)

    # x_pre[c, p, kt, s'] = x[b][c*CH+s', kt*128+p]
    xT = [np.ascontiguousarray(
        hidden_states[b].reshape(NCH, CH, KT, 128).transpose(0, 3, 2, 1)
        ).astype(NPBF16) for b in range(B)]
    NPF8 = ml_dtypes.float8_e4m3
    xT8 = [np.clip(np.asarray(t, np.float32) * 2.0, -240.0, 240.0).astype(NPF8)
           for t in xT] if causal else None
    cosT = [np.ascontiguousarray(cos[b].T).astype(NPBF16) for b in range(B)]
    ss = np.concatenate([-sin[:, :, :HD // 2], sin[:, :, HD // 2:]], axis=-1)
    ssT = [np.ascontiguousarray(ss[b].T).astype(NPBF16) for b in range(B)]
    emaskT16 = None

    # effective weights: W_eff[outdim, h] = W[outdim, h] + s*(A @ B).T[outdim, h]
    qw_eff = q_w + LORA_SCALE * (q_A @ q_B).T
    kw_eff = k_w + LORA_SCALE * (k_A @ k_B).T
    vw_eff = v_w + LORA_SCALE * (v_A @ v_B).T

    in_maps = []
    for c in range(NCORES):
        b, g = divmod(c, KVH)
        qsl = slice(QD * g, QD * (g + 1))
        ksl = slice(HD * g, HD * (g + 1))
        w_cat = np.concatenate([qw_eff[qsl], kw_eff[ksl], vw_eff[ksl]], axis=0)
        # w_pre[p, t, kt, o] = w_cat[t*128+o, kt*128+p]
        wT_c = w_cat.reshape(NT, 128, KT, 128).transpose(3, 0, 2, 1)
        wT_b = np.ascontiguousarray(wT_c).astype(NPBF16)
        # v bias handled on host: after softmax-normalization its contribution
        # to the output is the constant row o_w @ vb_o (added post-gather)
        bias_cat = np.concatenate([q_b[qsl], k_b[ksl],
                                   np.zeros(HD, f32)]).astype(f32)
        bias_cols = bias_cat.reshape(NT, 128).T  # [128, NT]
        swap_idx = np.concatenate([np.arange(64, 128), np.arange(0, 64)])
        biasT_c = np.ascontiguousarray(
            np.concatenate([bias_cols, bias_cols[swap_idx]], axis=1))  # [128, 2*NT]
        owT_c = o_w[:, qsl].T
        m = {
            "xT": xT[b],
            "wT": wT_b,
            "biasT": biasT_c,
            "cachetag": np.zeros((1, (K_TAG_INT % 97) + 1), f32),
            "cosT": cosT[b],
            "ssT": ssT[b],
            "owT": np.ascontiguousarray(owT_c).astype(NPBF16),
        }
        if causal:
            m["xT8"] = xT8[b]
            # fp8 weights at 32x scale: q0..q3,k straight; v as hi+lo split
            ws = np.asarray(wT_b, np.float32) * 32.0
            qk_hi = np.clip(ws[:, :NT - 1], -240.0, 240.0).astype(NPF8)
            v_hi = np.clip(ws[:, NT - 1:NT], -240.0, 240.0).astype(NPF8)
            v_lo = np.clip(ws[:, NT - 1:NT] - v_hi.astype(np.float32),
                           -240.0, 240.0).astype(NPF8)
            m["w8T"] = np.ascontiguousarray(
                np.concatenate([qk_hi, v_hi, v_lo], axis=1))
        if not causal and any(grid[i][j] == MASKED for i in range(NKS) for j in range(NCH)):
            if emaskT16 is None:
                emaskT16 = [e.astype(NPBF16) for e in emaskT]
            m["emaskT"] = emaskT16[b]
        in_maps.append(m)

    res = run_bass_kernel_spmd(nc, in_maps, core_ids=list(range(NCORES)))
    outs = [np.asarray(r["out_p"], dtype=f32) for r in res.results]
    # v-bias contribution: softmax rows sum to 1, so the +v_b term passes
    # through attention unchanged and adds o_w @ vb_o to every output row
    vb_o = np.empty(NH * HD, f32)
    for g in range(KVH):
        vb_o[QD * g:QD * (g + 1)] = np.tile(v_b[HD * g:HD * (g + 1)], GQ)
    delta = (o_w.astype(f32) @ vb_o)[None, :]  # [1, H]
    full = np.empty((B, S, H), f32)
    for b in range(B):
        full[b] = outs[KVH * b]
        for g in range(1, KVH):
            full[b] += outs[KVH * b + g]
        full[b] += delta
    return full

